# revision 4
# baseline (speedup 1.0000x reference)
"""DeepDFA on Trainium2 (Bass/Tile), 8-core data-parallel.

The reference computes a sequential one-hot DFA recurrence:
    s_{t+1} = s_t @ trans_prob[a_t];  r_t = s_{t+1} @ accepting_matrix
with trans_prob a *deterministic* one-hot transition tensor and s0 one-hot.
Every intermediate state is therefore exactly one-hot and the whole
computation collapses to integer table lookups
    s' = T1[a, s],   rewards[t] = one_hot(accbit[s_{t+1}], 2)
which this kernel evaluates exactly (float outputs are exact 0.0/1.0).

Device algorithm (per core, batch-sharded 4096 -> 8 x 512):
  * Three DFA steps are composed into one table row. T3[(a0,a1,a2), s]
    packs 8*s3 + b1 + 2*b2 + 4*b3 (s3 = state after 3 steps, b_r = accept
    bit after step r) as 64 x u32 = 256B rows in HBM. Rows for every
    (sequence, block) are bulk-gathered with dma_gather - parallel work,
    overlapped with the chain. A separate 1024-row pair table handles the
    last 2 steps (512 = 3*170 + 2).
  * The irreducibly sequential part runs on the vector engine as a
    171-step dependent chain x 4 independent 128-batch chunks: one
    scalar_tensor_tensor selects row[sigma] via iota compare and
    accumulates the packed value into a trace; one fused tensor_scalar
    (x*0.125 - 0.4375 -> int32, round-to-nearest) recovers sigma.
  * Accept bits are unpacked from the trace in bulk and written as the
    [B, L, 2] one-hot rewards; s_final is an iota compare on final sigma.
"""
import numpy as np

P = 128
NCHUNK = 4
B_CORE = P * NCHUNK      # 512 sequences per core
N_CORES = 8
BATCH = B_CORE * N_CORES
L = 512
NA = 32
NS = 64

_cache = {}


def _build_tables(trans_prob, accepting_matrix):
    t1 = np.argmax(trans_prob, axis=2).astype(np.int64)            # [A, S]
    accbit = np.argmax(accepting_matrix, axis=1).astype(np.int64)  # [S]

    s1 = t1                                                        # [a0, S]
    b1 = accbit[s1]
    s1e = np.broadcast_to(s1[:, None, :], (NA, NA, NS))
    s2 = t1[np.arange(NA)[None, :, None], s1e]                     # [a0,a1,S]
    b2 = accbit[s2]
    b1e = np.broadcast_to(b1[:, None, :], (NA, NA, NS))
    t2val = 16 * s2 + b1e + 2 * b2

    s2e = np.broadcast_to(s2[:, :, None, :], (NA, NA, NA, NS))
    s3 = t1[np.arange(NA)[None, None, :, None], s2e]               # [a0,a1,a2,S]
    b3 = accbit[s3]
    t3val = (16 * s3 + np.broadcast_to(b1e[:, :, None, :], s3.shape)
             + 2 * np.broadcast_to(b2[:, :, None, :], s3.shape) + 4 * b3)

    table3 = np.ascontiguousarray(t3val.reshape(NA**3, NS), dtype=np.uint32)
    table2 = np.ascontiguousarray(t2val.reshape(NA**2, NS), dtype=np.uint32)
    return table3, table2


def _make_windows(l_total):
    n3 = (l_total - 2) // 3
    assert l_total - 3 * n3 == 2
    windows = []
    j = 0
    while j < n3:
        w = min(16, n3 - j)
        windows.append((j, w))
        j += w
    return n3, windows


def _host_indices(action_shard, l_total):
    """alpha indices for one core in dma_gather wrap layout."""
    n3, windows = _make_windows(l_total)
    a = action_shard.astype(np.int64)
    alpha3 = a[:, 0:3 * n3:3] * 1024 + a[:, 1:3 * n3:3] * 32 + a[:, 2:3 * n3:3]
    alphap = a[:, 3 * n3] * 32 + a[:, 3 * n3 + 1]

    segs = []
    for (j0, w) in windows:
        blk = alpha3[:, j0:j0 + w].reshape(NCHUNK, P, w)       # [q, p, w]
        segs.append(np.transpose(blk, (2, 0, 1)).reshape(-1))  # [(w q p)]
    idx3_flat = np.concatenate(segs)
    wrap3 = idx3_flat.reshape(-1, 16).T.astype(np.int16)
    idx3 = np.tile(wrap3, (8, 1))                              # [128, slots]

    ordp = alphap.reshape(NCHUNK, P).reshape(-1)               # [(q p)]
    wrapp = ordp.reshape(32, 16).T.astype(np.int16)
    idxp = np.tile(wrapp, (8, 1))
    return idx3, idxp


def _build_kernel(l_total):
    import concourse.bacc as bacc
    import concourse.mybir as mybir
    from concourse.tile import TileContext

    n3, windows = _make_windows(l_total)
    total_slots = (n3 * NCHUNK * P) // 16
    nblocks = n3 + 1

    nc = bacc.Bacc("TRN2", target_bir_lowering=False,
                   dynamic_dma_scratch_size=1 << 16, num_swdge_queues=4)

    table3_d = nc.dram_tensor("table3", [NA**3, NS], mybir.dt.uint32,
                              kind="ExternalInput")
    table2_d = nc.dram_tensor("table2", [NA**2, NS], mybir.dt.uint32,
                              kind="ExternalInput")
    idx3_d = nc.dram_tensor("idx3", [P, total_slots], mybir.dt.int16,
                            kind="ExternalInput")
    idxp_d = nc.dram_tensor("idxp", [P, 32], mybir.dt.int16,
                            kind="ExternalInput")
    rewards_d = nc.dram_tensor("rewards", [B_CORE, l_total * 2],
                               mybir.dt.float32, kind="ExternalOutput")
    sfinal_d = nc.dram_tensor("sfinal", [B_CORE, NS], mybir.dt.float32,
                              kind="ExternalOutput")

    with TileContext(nc) as tc:
        with tc.tile_pool(name="pool", bufs=1) as pool:
            idx3 = pool.tile([P, total_slots], mybir.dt.int16)
            nc.sync.dma_start(idx3[:, :], idx3_d[:, :])
            idxp = pool.tile([P, 32], mybir.dt.int16)
            nc.sync.dma_start(idxp[:, :], idxp_d[:, :])

            iota_i = pool.tile([P, NS], mybir.dt.int32)
            nc.gpsimd.iota(iota_i[:, :], [[1, NS]], channel_multiplier=0)
            iota_f = pool.tile([P, NS], mybir.dt.float32)
            nc.vector.tensor_copy(iota_f[:, :], iota_i[:, :])

            sig = [pool.tile([P, 1], mybir.dt.int32, name=f"sig{q}",
                             tag=f"sig{q}") for q in range(NCHUNK)]
            for q in range(NCHUNK):
                nc.vector.memset(sig[q][:, :], 0)

            trace = pool.tile([P, NCHUNK, nblocks], mybir.dt.float32)
            scratch = [pool.tile([P, NS], mybir.dt.float32, name=f"scr{q}",
                                 tag=f"scr{q}") for q in range(NCHUNK)]

            prows = pool.tile([P, NCHUNK * NS], mybir.dt.uint32)
            nc.gpsimd.dma_gather(
                out_ap=prows[:, :].rearrange("p (k d) -> p k d", d=NS),
                in_ap=table2_d[:, :],
                idxs_ap=idxp[:, :],
                num_idxs=NCHUNK * P,
                num_idxs_reg=NCHUNK * P,
                elem_size=NS,
                single_packet=False,
                queue_num=3,
            )

            with tc.tile_pool(name="rowpool", bufs=3) as rowpool:
                for wi, (j0, w) in enumerate(windows):
                    nrow = w * NCHUNK
                    rows = rowpool.tile([P, nrow * NS], mybir.dt.uint32,
                                        name="rows", tag="rows",
                                        padded_shape=[P, 16 * NCHUNK * NS])
                    slot0 = (j0 * NCHUNK * P) // 16
                    nc.gpsimd.dma_gather(
                        out_ap=rows[:, :].rearrange("p (k d) -> p k d", d=NS),
                        in_ap=table3_d[:, :],
                        idxs_ap=idx3[:, slot0:slot0 + (nrow * P) // 16],
                        num_idxs=nrow * P,
                        num_idxs_reg=nrow * P,
                        elem_size=NS,
                        single_packet=False,
                        queue_num=wi % 3,
                    )
                    for jj in range(w):
                        j = j0 + jj
                        for q in range(NCHUNK):
                            r0 = (jj * NCHUNK + q) * NS
                            nc.vector.scalar_tensor_tensor(
                                out=scratch[q][:, :],
                                in0=iota_f[:, :],
                                scalar=sig[q][:, :],
                                in1=rows[:, r0:r0 + NS],
                                op0=mybir.AluOpType.is_equal,
                                op1=mybir.AluOpType.mult,
                                accum_out=trace[:, q, j:j + 1],
                            )
                            nc.scalar.activation(
                                sig[q][:, :], trace[:, q, j:j + 1],
                                mybir.ActivationFunctionType.Identity,
                                scale=0.0625,
                            )

            for q in range(NCHUNK):
                nc.vector.scalar_tensor_tensor(
                    out=scratch[q][:, :],
                    in0=iota_f[:, :],
                    scalar=sig[q][:, :],
                    in1=prows[:, q * NS:(q + 1) * NS],
                    op0=mybir.AluOpType.is_equal,
                    op1=mybir.AluOpType.mult,
                    accum_out=trace[:, q, n3:n3 + 1],
                )
                nc.scalar.activation(
                    sig[q][:, :], trace[:, q, n3:n3 + 1],
                    mybir.ActivationFunctionType.Identity,
                    scale=0.0625,
                )

            # bulk unpack: v = 16*s + b1 + 2*b2 + 4*b3
            nb = nblocks
            s_all = pool.tile([P, NCHUNK, nb], mybir.dt.int32)
            nc.vector.tensor_scalar(
                out=s_all[:, :, :], in0=trace[:, :, :],
                scalar1=0.0625, scalar2=None,
                op0=mybir.AluOpType.mult,
            )
            w_all = pool.tile([P, NCHUNK, nb], mybir.dt.float32)
            nc.vector.scalar_tensor_tensor(
                out=w_all[:, :, :], in0=s_all[:, :, :], scalar=-16.0,
                in1=trace[:, :, :],
                op0=mybir.AluOpType.mult, op1=mybir.AluOpType.add,
            )
            b3 = pool.tile([P, NCHUNK, nb], mybir.dt.float32)
            nc.vector.tensor_scalar(
                out=b3[:, :, :], in0=w_all[:, :, :], scalar1=4.0, scalar2=None,
                op0=mybir.AluOpType.is_ge,
            )
            w2 = pool.tile([P, NCHUNK, nb], mybir.dt.float32)
            nc.vector.scalar_tensor_tensor(
                out=w2[:, :, :], in0=b3[:, :, :], scalar=-4.0,
                in1=w_all[:, :, :],
                op0=mybir.AluOpType.mult, op1=mybir.AluOpType.add,
            )
            b2 = pool.tile([P, NCHUNK, nb], mybir.dt.float32)
            nc.vector.tensor_scalar(
                out=b2[:, :, :], in0=w2[:, :, :], scalar1=2.0, scalar2=None,
                op0=mybir.AluOpType.is_ge,
            )
            b1 = pool.tile([P, NCHUNK, nb], mybir.dt.float32)
            nc.vector.scalar_tensor_tensor(
                out=b1[:, :, :], in0=b2[:, :, :], scalar=-2.0,
                in1=w2[:, :, :],
                op0=mybir.AluOpType.mult, op1=mybir.AluOpType.add,
            )

            rew = pool.tile([P, NCHUNK, l_total, 2], mybir.dt.float32)
            bits = [b1, b2, b3]
            for r in range(3):
                src = bits[r][:, :, 0:n3]
                nc.vector.tensor_copy(rew[:, :, r:3 * n3 + r:3, 1], src)
                nc.vector.tensor_scalar(
                    out=rew[:, :, r:3 * n3 + r:3, 0], in0=src,
                    scalar1=-1.0, scalar2=1.0,
                    op0=mybir.AluOpType.mult, op1=mybir.AluOpType.add,
                )
            for r in range(2):
                src = bits[r][:, :, n3:n3 + 1]
                t = 3 * n3 + r
                nc.vector.tensor_copy(rew[:, :, t:t + 1, 1], src)
                nc.vector.tensor_scalar(
                    out=rew[:, :, t:t + 1, 0], in0=src,
                    scalar1=-1.0, scalar2=1.0,
                    op0=mybir.AluOpType.mult, op1=mybir.AluOpType.add,
                )

            sfin = pool.tile([P, NCHUNK, NS], mybir.dt.float32)
            sig_f = pool.tile([P, NCHUNK], mybir.dt.float32)
            for q in range(NCHUNK):
                nc.vector.tensor_copy(sig_f[:, q:q + 1], sig[q][:, :])
            for q in range(NCHUNK):
                nc.vector.tensor_scalar(
                    out=sfin[:, q, :], in0=iota_f[:, :],
                    scalar1=sig_f[:, q:q + 1],
                    scalar2=None, op0=mybir.AluOpType.is_equal,
                )

            rew_view = rewards_d[:, :].rearrange("(q p) x -> p q x", p=P)
            nc.sync.dma_start(
                rew_view,
                rew[:, :, :, :].rearrange("p q t two -> p q (t two)"))
            sf_view = sfinal_d[:, :].rearrange("(q p) x -> p q x", p=P)
            nc.sync.dma_start(sf_view, sfin[:, :, :])

    nc.compile()
    return nc


def _get_kernel(l_total):
    if l_total not in _cache:
        _cache[l_total] = _build_kernel(l_total)
    return _cache[l_total]


def kernel(action_seq, trans_prob, accepting_matrix):
    from concourse.bass_utils import run_bass_kernel_spmd

    action_seq = np.asarray(action_seq)
    trans_prob = np.asarray(trans_prob)
    accepting_matrix = np.asarray(accepting_matrix)
    batch, l_total = action_seq.shape
    assert batch == BATCH and l_total == L, (batch, l_total)

    table3, table2 = _build_tables(trans_prob, accepting_matrix)
    nc = _get_kernel(l_total)

    in_maps = []
    for c in range(N_CORES):
        shard = action_seq[c * B_CORE:(c + 1) * B_CORE]
        idx3, idxp = _host_indices(shard, l_total)
        in_maps.append({
            "table3": table3,
            "table2": table2,
            "idx3": idx3,
            "idxp": idxp,
        })

    res = run_bass_kernel_spmd(nc, in_maps, core_ids=list(range(N_CORES)))

    rewards = np.concatenate(
        [r["rewards"].reshape(B_CORE, l_total, 2) for r in res.results], axis=0)
    s_final_idx = np.concatenate(
        [r["sfinal"] for r in res.results], axis=0)
    rewards = rewards.astype(trans_prob.dtype, copy=False)
    s_final = s_final_idx.astype(trans_prob.dtype, copy=False)
    return rewards, s_final


# revision 6
# speedup vs baseline: 1.0381x; 1.0381x over previous
"""DeepDFA on Trainium2 (Bass/Tile), 8-core data-parallel.

The reference computes a sequential one-hot DFA recurrence:
    s_{t+1} = s_t @ trans_prob[a_t];  r_t = s_{t+1} @ accepting_matrix
with trans_prob a *deterministic* one-hot transition tensor and s0 one-hot.
Every intermediate state is therefore exactly one-hot and the whole
computation collapses to integer table lookups
    s' = T1[a, s],   rewards[t] = one_hot(accbit[s_{t+1}], 2)
which this kernel evaluates exactly (float outputs are exact 0.0/1.0).

Device algorithm (per core, batch-sharded 4096 -> 8 x 512):
  * Three DFA steps are composed into one table row. T3[(a0,a1,a2), s]
    packs 16*s3 + b1 + 2*b2 + 4*b3 (s3 = state after 3 steps, b_r = accept
    bit after step r) as 64 x u32 = 256B rows in HBM. Rows for every
    (sequence, block) are bulk-gathered with dma_gather - parallel work,
    overlapped with the chain. A separate 1024-row pair table handles the
    last 2 steps (512 = 3*170 + 2).
  * The irreducibly sequential part runs on the vector engine as a
    171-step dependent chain x 4 independent 128-batch chunks: one
    scalar_tensor_tensor selects row[sigma] via iota compare and
    accumulates the packed value into a trace; one scalar-engine
    activation (x*0.0625 -> int32; exact under round or truncate since
    bits/16 < 0.5) recovers sigma, overlapping the two engines.
  * Accept bits are unpacked from the trace in bulk and written as the
    [B, L, 2] one-hot rewards; s_final is an iota compare on final sigma.
"""
import numpy as np

P = 128
NCHUNK = 4
B_CORE = P * NCHUNK      # 512 sequences per core
N_CORES = 8
BATCH = B_CORE * N_CORES
L = 512
NA = 32
NS = 64

_cache = {}


def _build_tables(trans_prob, accepting_matrix):
    t1 = np.argmax(trans_prob, axis=2).astype(np.int64)            # [A, S]
    accbit = np.argmax(accepting_matrix, axis=1).astype(np.int64)  # [S]

    s1 = t1                                                        # [a0, S]
    b1 = accbit[s1]
    s1e = np.broadcast_to(s1[:, None, :], (NA, NA, NS))
    s2 = t1[np.arange(NA)[None, :, None], s1e]                     # [a0,a1,S]
    b2 = accbit[s2]
    b1e = np.broadcast_to(b1[:, None, :], (NA, NA, NS))
    t2val = 16 * s2 + b1e + 2 * b2

    s2e = np.broadcast_to(s2[:, :, None, :], (NA, NA, NA, NS))
    s3 = t1[np.arange(NA)[None, None, :, None], s2e]               # [a0,a1,a2,S]
    b3 = accbit[s3]
    t3val = (16 * s3 + np.broadcast_to(b1e[:, :, None, :], s3.shape)
             + 2 * np.broadcast_to(b2[:, :, None, :], s3.shape) + 4 * b3)

    table3 = np.ascontiguousarray(t3val.reshape(NA**3, NS), dtype=np.uint32)
    table2 = np.ascontiguousarray(t2val.reshape(NA**2, NS), dtype=np.uint32)
    return table3, table2


def _make_windows(l_total):
    n3 = (l_total - 2) // 3
    assert l_total - 3 * n3 == 2
    windows = []
    j = 0
    while j < n3:
        w = min(16, n3 - j)
        windows.append((j, w))
        j += w
    return n3, windows


def _host_indices(action_shard, l_total):
    """alpha indices for one core in dma_gather wrap layout."""
    n3, windows = _make_windows(l_total)
    a = action_shard.astype(np.int64)
    alpha3 = a[:, 0:3 * n3:3] * 1024 + a[:, 1:3 * n3:3] * 32 + a[:, 2:3 * n3:3]
    alphap = a[:, 3 * n3] * 32 + a[:, 3 * n3 + 1]

    segs = []
    for (j0, w) in windows:
        blk = alpha3[:, j0:j0 + w].reshape(NCHUNK, P, w)       # [q, p, w]
        segs.append(np.transpose(blk, (2, 0, 1)).reshape(-1))  # [(w q p)]
    idx3_flat = np.concatenate(segs)
    wrap3 = idx3_flat.reshape(-1, 16).T.astype(np.int16)
    idx3 = np.tile(wrap3, (8, 1))                              # [128, slots]

    ordp = alphap.reshape(NCHUNK, P).reshape(-1)               # [(q p)]
    wrapp = ordp.reshape(32, 16).T.astype(np.int16)
    idxp = np.tile(wrapp, (8, 1))
    return idx3, idxp


def _build_kernel(l_total):
    import concourse.bacc as bacc
    import concourse.mybir as mybir
    from concourse.tile import TileContext

    n3, windows = _make_windows(l_total)
    total_slots = (n3 * NCHUNK * P) // 16
    nblocks = n3 + 1

    nc = bacc.Bacc("TRN2", target_bir_lowering=False,
                   dynamic_dma_scratch_size=1 << 16, num_swdge_queues=4)

    table3_d = nc.dram_tensor("table3", [NA**3, NS], mybir.dt.uint32,
                              kind="ExternalInput")
    table2_d = nc.dram_tensor("table2", [NA**2, NS], mybir.dt.uint32,
                              kind="ExternalInput")
    idx3_d = nc.dram_tensor("idx3", [P, total_slots], mybir.dt.int16,
                            kind="ExternalInput")
    idxp_d = nc.dram_tensor("idxp", [P, 32], mybir.dt.int16,
                            kind="ExternalInput")
    rewards_d = nc.dram_tensor("rewards", [B_CORE, l_total * 2],
                               mybir.dt.float32, kind="ExternalOutput")
    sfinal_d = nc.dram_tensor("sfinal", [B_CORE, NS], mybir.dt.float32,
                              kind="ExternalOutput")

    with TileContext(nc) as tc:
        with tc.tile_pool(name="pool", bufs=1) as pool:
            idx3 = pool.tile([P, total_slots], mybir.dt.int16)
            nc.sync.dma_start(idx3[:, :], idx3_d[:, :])
            idxp = pool.tile([P, 32], mybir.dt.int16)
            nc.sync.dma_start(idxp[:, :], idxp_d[:, :])

            iota_i = pool.tile([P, NS], mybir.dt.int32)
            nc.gpsimd.iota(iota_i[:, :], [[1, NS]], channel_multiplier=0)
            iota_f = pool.tile([P, NS], mybir.dt.float32)
            nc.vector.tensor_copy(iota_f[:, :], iota_i[:, :])

            sig = [pool.tile([P, 1], mybir.dt.int32, name=f"sig{q}",
                             tag=f"sig{q}") for q in range(NCHUNK)]
            for q in range(NCHUNK):
                nc.vector.memset(sig[q][:, :], 0)

            trace = pool.tile([P, NCHUNK, nblocks], mybir.dt.float32)
            scratch = [pool.tile([P, NS], mybir.dt.float32, name=f"scr{q}",
                                 tag=f"scr{q}") for q in range(NCHUNK)]

            prows = pool.tile([P, NCHUNK * NS], mybir.dt.uint32)
            nc.gpsimd.dma_gather(
                out_ap=prows[:, :].rearrange("p (k d) -> p k d", d=NS),
                in_ap=table2_d[:, :],
                idxs_ap=idxp[:, :],
                num_idxs=NCHUNK * P,
                num_idxs_reg=NCHUNK * P,
                elem_size=NS,
                single_packet=False,
                queue_num=3,
            )

            with tc.tile_pool(name="rowpool", bufs=3) as rowpool:
                for wi, (j0, w) in enumerate(windows):
                    nrow = w * NCHUNK
                    rows = rowpool.tile([P, nrow * NS], mybir.dt.uint32,
                                        name="rows", tag="rows",
                                        padded_shape=[P, 16 * NCHUNK * NS])
                    slot0 = (j0 * NCHUNK * P) // 16
                    # split each window across two SWDGE queues: the cost
                    # model serializes all queues on one DMA-engine pool, but
                    # real SDMA engines are per-queue, so finer spreading
                    # raises gather parallelism on HW.
                    half = nrow // 2
                    for h in range(2):
                        lo, hi = h * half, (nrow if h else half)
                        nslot = ((hi - lo) * P) // 16
                        sl = slot0 + (lo * P) // 16
                        nc.gpsimd.dma_gather(
                            out_ap=rows[:, lo * NS:hi * NS]
                                .rearrange("p (k d) -> p k d", d=NS),
                            in_ap=table3_d[:, :],
                            idxs_ap=idx3[:, sl:sl + nslot],
                            num_idxs=(hi - lo) * P,
                            num_idxs_reg=(hi - lo) * P,
                            elem_size=NS,
                            single_packet=False,
                            queue_num=(wi * 2 + h) % 4,
                        )
                    for jj in range(w):
                        j = j0 + jj
                        for q in range(NCHUNK):
                            r0 = (jj * NCHUNK + q) * NS
                            nc.vector.scalar_tensor_tensor(
                                out=scratch[q][:, :],
                                in0=iota_f[:, :],
                                scalar=sig[q][:, :],
                                in1=rows[:, r0:r0 + NS],
                                op0=mybir.AluOpType.is_equal,
                                op1=mybir.AluOpType.mult,
                                accum_out=trace[:, q, j:j + 1],
                            )
                            nc.scalar.activation(
                                sig[q][:, :], trace[:, q, j:j + 1],
                                mybir.ActivationFunctionType.Identity,
                                scale=0.0625,
                            )

            for q in range(NCHUNK):
                nc.vector.scalar_tensor_tensor(
                    out=scratch[q][:, :],
                    in0=iota_f[:, :],
                    scalar=sig[q][:, :],
                    in1=prows[:, q * NS:(q + 1) * NS],
                    op0=mybir.AluOpType.is_equal,
                    op1=mybir.AluOpType.mult,
                    accum_out=trace[:, q, n3:n3 + 1],
                )
                nc.scalar.activation(
                    sig[q][:, :], trace[:, q, n3:n3 + 1],
                    mybir.ActivationFunctionType.Identity,
                    scale=0.0625,
                )

            # bulk unpack: v = 16*s + b1 + 2*b2 + 4*b3
            nb = nblocks
            s_all = pool.tile([P, NCHUNK, nb], mybir.dt.int32)
            nc.vector.tensor_scalar(
                out=s_all[:, :, :], in0=trace[:, :, :],
                scalar1=0.0625, scalar2=None,
                op0=mybir.AluOpType.mult,
            )
            w_all = pool.tile([P, NCHUNK, nb], mybir.dt.float32)
            nc.vector.scalar_tensor_tensor(
                out=w_all[:, :, :], in0=s_all[:, :, :], scalar=-16.0,
                in1=trace[:, :, :],
                op0=mybir.AluOpType.mult, op1=mybir.AluOpType.add,
            )
            b3 = pool.tile([P, NCHUNK, nb], mybir.dt.float32)
            nc.vector.tensor_scalar(
                out=b3[:, :, :], in0=w_all[:, :, :], scalar1=4.0, scalar2=None,
                op0=mybir.AluOpType.is_ge,
            )
            w2 = pool.tile([P, NCHUNK, nb], mybir.dt.float32)
            nc.vector.scalar_tensor_tensor(
                out=w2[:, :, :], in0=b3[:, :, :], scalar=-4.0,
                in1=w_all[:, :, :],
                op0=mybir.AluOpType.mult, op1=mybir.AluOpType.add,
            )
            b2 = pool.tile([P, NCHUNK, nb], mybir.dt.float32)
            nc.vector.tensor_scalar(
                out=b2[:, :, :], in0=w2[:, :, :], scalar1=2.0, scalar2=None,
                op0=mybir.AluOpType.is_ge,
            )
            b1 = pool.tile([P, NCHUNK, nb], mybir.dt.float32)
            nc.vector.scalar_tensor_tensor(
                out=b1[:, :, :], in0=b2[:, :, :], scalar=-2.0,
                in1=w2[:, :, :],
                op0=mybir.AluOpType.mult, op1=mybir.AluOpType.add,
            )

            rew = pool.tile([P, NCHUNK, l_total, 2], mybir.dt.float32)
            bits = [b1, b2, b3]
            for r in range(3):
                src = bits[r][:, :, 0:n3]
                nc.vector.tensor_copy(rew[:, :, r:3 * n3 + r:3, 1], src)
                nc.vector.tensor_scalar(
                    out=rew[:, :, r:3 * n3 + r:3, 0], in0=src,
                    scalar1=-1.0, scalar2=1.0,
                    op0=mybir.AluOpType.mult, op1=mybir.AluOpType.add,
                )
            for r in range(2):
                src = bits[r][:, :, n3:n3 + 1]
                t = 3 * n3 + r
                nc.vector.tensor_copy(rew[:, :, t:t + 1, 1], src)
                nc.vector.tensor_scalar(
                    out=rew[:, :, t:t + 1, 0], in0=src,
                    scalar1=-1.0, scalar2=1.0,
                    op0=mybir.AluOpType.mult, op1=mybir.AluOpType.add,
                )

            sfin = pool.tile([P, NCHUNK, NS], mybir.dt.float32)
            sig_f = pool.tile([P, NCHUNK], mybir.dt.float32)
            for q in range(NCHUNK):
                nc.vector.tensor_copy(sig_f[:, q:q + 1], sig[q][:, :])
            for q in range(NCHUNK):
                nc.vector.tensor_scalar(
                    out=sfin[:, q, :], in0=iota_f[:, :],
                    scalar1=sig_f[:, q:q + 1],
                    scalar2=None, op0=mybir.AluOpType.is_equal,
                )

            rew_view = rewards_d[:, :].rearrange("(q p) x -> p q x", p=P)
            nc.sync.dma_start(
                rew_view,
                rew[:, :, :, :].rearrange("p q t two -> p q (t two)"))
            sf_view = sfinal_d[:, :].rearrange("(q p) x -> p q x", p=P)
            nc.sync.dma_start(sf_view, sfin[:, :, :])

    nc.compile()
    return nc


def _get_kernel(l_total):
    if l_total not in _cache:
        _cache[l_total] = _build_kernel(l_total)
    return _cache[l_total]


def kernel(action_seq, trans_prob, accepting_matrix):
    from concourse.bass_utils import run_bass_kernel_spmd

    action_seq = np.asarray(action_seq)
    trans_prob = np.asarray(trans_prob)
    accepting_matrix = np.asarray(accepting_matrix)
    batch, l_total = action_seq.shape
    assert batch == BATCH and l_total == L, (batch, l_total)

    table3, table2 = _build_tables(trans_prob, accepting_matrix)
    nc = _get_kernel(l_total)

    in_maps = []
    for c in range(N_CORES):
        shard = action_seq[c * B_CORE:(c + 1) * B_CORE]
        idx3, idxp = _host_indices(shard, l_total)
        in_maps.append({
            "table3": table3,
            "table2": table2,
            "idx3": idx3,
            "idxp": idxp,
        })

    res = run_bass_kernel_spmd(nc, in_maps, core_ids=list(range(N_CORES)))

    rewards = np.concatenate(
        [r["rewards"].reshape(B_CORE, l_total, 2) for r in res.results], axis=0)
    s_final_idx = np.concatenate(
        [r["sfinal"] for r in res.results], axis=0)
    rewards = rewards.astype(trans_prob.dtype, copy=False)
    s_final = s_final_idx.astype(trans_prob.dtype, copy=False)
    return rewards, s_final
